# revision 28
# baseline (speedup 1.0000x reference)
"""TRN2 Bass kernel for nn_CustomQLoRABigNet: 6 blocks x (3 QLoRA linears),
ReLU, residual, LayerNorm. Data-parallel over 8 NeuronCores (4096 rows each).

v4 design:
- All matmul operands f16 (PSUM f32). LoRA is folded into the dequantized
  weight on-chip: W' = dequant(q, s) + (lb @ la)^T, built with 16 small PE
  passes per layer; each layer is then one dense 1024x1024 matmul.
- chunk = 2048 columns per weight pass (2 chunks per core).
- 3-buffer hidden-state rotation; residuals are added in PSUM via
  identity / diag(gamma) matmuls (no vector-engine residual work).
- LayerNorm gamma/beta are folded away: gamma scales the next layer's
  weight during dequant (scalar_tensor_tensor, free) and the residual
  diag matmul; beta/gamma is carried inside the stored hidden state via
  a K=2 broadcast matmul ([ones; beta/gamma]^T @ [m*is; -1]). The LN
  apply is then just one mult + one sub per 128x512 tile on the DVE.
- LN stats via inline accumulating PE matmuls (lag-one-ot), inv-std via
  a raw Rsqrt activation (one act-table set -> no ACT_TABLE_LOAD thrash).
"""

import sys

sys.path.insert(0, "/opt/trn_rl_repo")

import numpy as np
import ml_dtypes

import concourse.bass as bass
from concourse import bacc, mybir
import concourse.tile as tile
from concourse.bass_utils import run_bass_kernel_spmd

f32 = mybir.dt.float32
f32r = mybir.dt.float32r
f16 = mybir.dt.float16
i8 = mybir.dt.int8
AF = mybir.ActivationFunctionType
Alu = mybir.AluOpType

N_CORES = 8
DIM = 1024
KT = 8  # 1024 / 128 partition tiles
NL = 18
RANK = 32
GROUP = 16
BATCH = 32768
RPC = BATCH // N_CORES  # rows per core
CHUNK = 2048  # columns (rows of x) processed per weight pass
NCH = RPC // CHUNK
NT = 512  # matmul moving free dim (one PSUM bank)
NTILES = CHUNK // NT
EPS = 1e-5
F16 = np.float16
USE_RSQRT = True

# layers whose input is a LayerNorm output (j0 of blocks 1..5): their
# weights absorb that LN's gamma
POST_LN = {3, 6, 9, 12, 15}


def build_kernel():
    nc = bacc.Bacc()

    x_d = nc.declare_dram_parameter("x_t", [128, KT, RPC], f16, False)
    wq_d = nc.declare_dram_parameter("wqc", [NL, 128, KT, DIM], i8, False)
    sr_d = nc.declare_dram_parameter("srep", [NL, 128, KT, DIM], f16, False)
    la_d = nc.declare_dram_parameter("la_r", [NL, RANK, KT, 128], f16, False)
    lb_d = nc.declare_dram_parameter("lb_r", [NL, RANK, DIM], f16, False)
    bi_d = nc.declare_dram_parameter("bias_pp", [128, NL, KT], f32, False)
    ga_d = nc.declare_dram_parameter("gamma_pp", [128, 5, KT], f32, False)
    bg_d = nc.declare_dram_parameter("bog_pp", [128, 5, KT], f32, False)
    gd_d = nc.declare_dram_parameter("gdiag", [5, 128, KT, 128], f16, False)
    on_d = nc.declare_dram_parameter("ones", [128, 128], f16, False)
    id_d = nc.declare_dram_parameter("ident", [128, 128], f16, False)
    y_d = nc.declare_dram_parameter("y_t", [128, KT, RPC], f16, True)

    with tile.TileContext(nc) as tc:
        with (
            tc.tile_pool(name="persist", bufs=1) as pp,
            tc.tile_pool(name="wts", bufs=2) as wp,
            tc.tile_pool(name="stage", bufs=1) as qp,
            tc.tile_pool(name="work", bufs=2) as sp,
            tc.tile_pool(name="ps", bufs=1, space="PSUM") as ps,
        ):
            h0 = pp.tile([128, KT, CHUNK], f16)
            h1 = pp.tile([128, KT, CHUNK], f16)
            h2 = pp.tile([128, KT, CHUNK], f16)
            hb = [h0, h1, h2]
            bias_t = pp.tile([128, NL, KT], f32)
            nc.sync.dma_start(bias_t[:, :, :], bi_d[:, :, :])
            gamma_t = pp.tile([128, 5, KT], f32)
            nc.sync.dma_start(gamma_t[:, :, :], ga_d[:, :, :])
            bog_t = pp.tile([128, 5, KT], f32)
            nc.sync.dma_start(bog_t[:, :, :], bg_d[:, :, :])
            ones_t = pp.tile([128, 128], f16)
            nc.sync.dma_start(ones_t[:, :], on_d[:, :])
            ident_t = pp.tile([128, 128], f16)
            nc.sync.dma_start(ident_t[:, :], id_d[:, :])
            ones_col = ones_t[:, 0:1]  # [128,1] stats lhsT
            ones_row = ones_t[0:1, :]  # [1,128] broadcast lhsT
            eps_t = pp.tile([1, 1], f32)
            nc.vector.memset(eps_t[:, :], EPS)

            def rsqrt(out, in_, bias_ap):
                # raw Rsqrt activation: bass's wrapper rejects it for
                # accuracy reasons irrelevant at this tolerance, and it
                # keeps the Scalar engine inside one act-func table set
                eng = nc.scalar
                ins = [
                    eng.lower_ap(in_),
                    eng.lower_ap(bias_ap),
                    mybir.ImmediateValue(dtype=mybir.dt.float32, value=1.0),
                    mybir.ImmediateValue(dtype=mybir.dt.float32, value=0.0),
                ]
                return eng.add_instruction(
                    mybir.InstActivation(
                        name=eng.bass.get_next_instruction_name(),
                        func=AF.Rsqrt,
                        ins=ins,
                        outs=[eng.lower_ap(out)],
                    )
                )

            def load_layer(li):
                blk, j = li // 3, li % 3
                wq_t = qp.tile([128, KT, DIM], i8, tag="wq")
                nc.sync.dma_start(wq_t[:, :, :], wq_d[li, :, :, :])
                sr_t = qp.tile([128, KT, DIM], f16, tag="sr")
                nc.sync.dma_start(sr_t[:, :, :], sr_d[li, :, :, :])
                la_t = wp.tile([RANK, KT, 128], f16, tag="la")
                nc.sync.dma_start(la_t[:, :, :], la_d[li, :, :, :])
                lb_t = wp.tile([RANK, DIM], f16, tag="lb")
                nc.sync.dma_start(lb_t[:, :], lb_d[li, :, :])
                gd_t = None
                if j == 2 and blk >= 1:
                    gd_t = wp.tile([128, KT, 128], f16, tag="gd")
                    nc.sync.dma_start(gd_t[:, :, :], gd_d[blk - 1, :, :, :])
                return wq_t, sr_t, la_t, lb_t, gd_t

            def dequant(tiles, li):
                # oh-major so the oh=0 half of w' is complete first
                wq_t, sr_t = tiles[0], tiles[1]
                wtmps = {}
                for oh in range(2):
                    for kt in range(KT):
                        oc = bass.ts(oh, NT)
                        wtmp = sp.tile([128, NT], f32, tag="wtmp", bufs=8)
                        if li in POST_LN:
                            g = gamma_t[:, li // 3 - 1, kt : kt + 1]
                            nc.vector.scalar_tensor_tensor(
                                wtmp[:, :], wq_t[:, kt, oc], g,
                                sr_t[:, kt, oc], Alu.mult, Alu.mult,
                            )
                        else:
                            nc.vector.tensor_mul(
                                wtmp[:, :], wq_t[:, kt, oc], sr_t[:, kt, oc]
                            )
                        wtmps[(kt, oh)] = wtmp
                return wtmps

            def build_w(tiles, wtmps, li):
                # W'[i, o] = gamma_i * ((q-8)*s + sum_r la[r,i]*lb[o,r])
                la_t, lb_t = tiles[2], tiles[3]
                w_t = wp.tile([128, KT, DIM], f16, tag="wt")
                for oh in range(2):
                    for kt in range(KT):
                        oc = bass.ts(oh, NT)
                        pb = ps.tile([128, NT], f32, tag="pb", bufs=2)
                        nc.tensor.matmul(
                            pb[:, :],
                            lhsT=la_t[:, kt, :],
                            rhs=lb_t[:, oc],
                            start=True,
                            stop=True,
                        )
                        if li in POST_LN:
                            g = gamma_t[:, li // 3 - 1, kt : kt + 1]
                            nc.vector.scalar_tensor_tensor(
                                w_t[:, kt, oc], pb[:, :], g,
                                wtmps[(kt, oh)][:, :], Alu.mult, Alu.add,
                            )
                        else:
                            nc.vector.tensor_add(
                                w_t[:, kt, oc], wtmps[(kt, oh)][:, :], pb[:, :]
                            )
                return w_t

            for c in range(NCH):
                inp0 = c % 3  # buffer receiving this chunk's x
                xbuf = hb[inp0]
                for kt in range(KT):
                    for nt in range(NTILES):
                        nc.sync.dma_start(
                            xbuf[:, kt, bass.ts(nt, NT)],
                            x_d[:, kt, bass.ts(c * NTILES + nt, NT)],
                        )

                if c == 0:
                    tiles = load_layer(0)
                    wtmps = dequant(tiles, 0)

                # LN finalization steps (one K=2 bcast matmul + DVE
                # mult/sub per kt) drip one per ot-chain across the
                # following tiles, so the PE stream is never gated by
                # the DVE apply ops (WAR on the rotating mib banks).
                pending = []

                def drip(n=1):
                    for _ in range(min(n, len(pending))):
                        pending.pop(0)()

                for li in range(NL):
                    blk, j = li // 3, li % 3
                    i = (inp0 + blk) % 3  # block input (residual) buffer
                    if j == 0:
                        src, dst = i, (i + 1) % 3
                    elif j == 1:
                        src, dst = (i + 1) % 3, (i + 2) % 3
                    else:
                        src, dst = (i + 2) % 3, (i + 1) % 3
                    h_in, h_out, r_buf = hb[src], hb[dst], hb[i]
                    ln_here = j == 2 and blk < 5
                    gd_t = tiles[4]

                    w_t = build_w(tiles, wtmps, li)
                    last = c == NCH - 1 and li == NL - 1
                    if not last:
                        tiles_nxt = load_layer((li + 1) % NL)

                    def emit_mains(nt):
                        cols = bass.ts(nt, NT)
                        sacc = []
                        s12 = None
                        if ln_here:
                            s12 = ps.tile([33, NT], f32, tag="s12", bufs=1)
                        for ot in range(KT):
                            y_ps = ps.tile([128, NT], f32, tag="y", bufs=2)
                            for kt in range(KT):
                                nc.tensor.matmul(
                                    y_ps[:, :],
                                    lhsT=w_t[:, kt, bass.ts(ot, 128)],
                                    rhs=h_in[:, kt, cols],
                                    start=(kt == 0),
                                    stop=(kt == KT - 1 and j != 2),
                                )
                            if j == 2:
                                # residual (diag(gamma) for blocks >= 1)
                                nc.tensor.matmul(
                                    y_ps[:, :],
                                    lhsT=ident_t[:, :] if blk == 0
                                    else gd_t[:, ot, :],
                                    rhs=r_buf[:, ot, cols],
                                    start=False,
                                    stop=True,
                                )
                            drip()
                            nc.scalar.activation(
                                h_out[:, ot, cols],
                                y_ps[:, :],
                                AF.Relu if j < 2 else AF.Identity,
                                bias=bias_t[:, li, ot : ot + 1],
                            )
                            if ln_here:
                                hsq = sp.tile([128, NT], f16, tag="hsq", bufs=3)
                                nc.scalar.square(hsq[:, :], h_out[:, ot, cols])
                                sacc.append((h_out[:, ot, cols], hsq))
                                # lag-one-ot stats so the PE never waits
                                # on the activation/square writes
                                if ot >= 1:
                                    ho_p, hq_p = sacc[ot - 1]
                                    nc.tensor.matmul(
                                        s12[0:1, :], lhsT=ones_col, rhs=ho_p,
                                        start=(ot == 1), stop=False,
                                    )
                                    nc.tensor.matmul(
                                        s12[32:33, :], lhsT=ones_col,
                                        rhs=hq_p[:, :],
                                        start=(ot == 1), stop=False,
                                    )
                        if ln_here:
                            ho_p, hq_p = sacc[KT - 1]
                            nc.tensor.matmul(
                                s12[0:1, :], lhsT=ones_col, rhs=ho_p,
                                start=False, stop=True,
                            )
                            nc.tensor.matmul(
                                s12[32:33, :], lhsT=ones_col, rhs=hq_p[:, :],
                                start=False, stop=True,
                            )
                            # inv-std chain (small ops, off the PE)
                            m_sb = sp.tile([1, NT], f32, tag="m", bufs=1)
                            nc.vector.tensor_scalar(
                                m_sb[:, :], s12[0:1, :], 1.0 / DIM, None,
                                Alu.mult,
                            )
                            msq = sp.tile([1, NT], f32, tag="msq", bufs=1)
                            nc.vector.tensor_mul(msq[:, :], m_sb[:, :], m_sb[:, :])
                            ve = sp.tile([1, NT], f32, tag="ve", bufs=1)
                            nc.vector.scalar_tensor_tensor(
                                ve[:, :], s12[32:33, :], 1.0 / DIM, msq[:, :],
                                Alu.mult, Alu.subtract,
                            )
                            is_sb = sp.tile([1, NT], f16, tag="isb", bufs=2)
                            if USE_RSQRT:
                                rsqrt(is_sb[:, :], ve[:, :], eps_t[:, :])
                            else:
                                lnv = sp.tile([1, NT], f32, tag="lnv", bufs=1)
                                nc.scalar.activation(
                                    lnv[:, :], ve[:, :], AF.Ln, bias=eps_t[:, :]
                                )
                                nc.scalar.activation(
                                    is_sb[:, :], lnv[:, :], AF.Exp, scale=-0.5
                                )
                            mis = sp.tile([1, NT], f16, tag="mis", bufs=2)
                            nc.vector.tensor_mul(
                                mis[:, :], m_sb[:, :], is_sb[:, :]
                            )

                            boxes = []

                            def step(kt, cols=cols, is_sb=is_sb, mis=mis,
                                     blk=blk, h_out=h_out, boxes=boxes):
                                if kt == 0:
                                    ib = ps.tile([128, NT], f32, tag="ib",
                                                 bufs=2)
                                    nc.tensor.matmul(
                                        ib[:, :], lhsT=ones_row,
                                        rhs=is_sb[:, :],
                                        start=True, stop=True,
                                    )
                                    mib = ps.tile([128, NT], f32, tag="mib",
                                                  bufs=1)
                                    nc.tensor.matmul(
                                        mib[:, :], lhsT=ones_row,
                                        rhs=mis[:, :],
                                        start=True, stop=True,
                                    )
                                    # free the mib bank immediately
                                    mib_sb = sp.tile([128, NT], f32r,
                                                     tag="mibs", bufs=2)
                                    nc.scalar.copy(mib_sb[:, :], mib[:, :])
                                    boxes.extend([ib, mib_sb])
                                ib, mib_sb = boxes
                                nc.vector.tensor_mul(
                                    h_out[:, kt, cols], h_out[:, kt, cols],
                                    ib[:, :],
                                )
                                # h = (h*ib + beta/gamma[p]) - m*is[n]
                                nc.vector.scalar_tensor_tensor(
                                    h_out[:, kt, cols], h_out[:, kt, cols],
                                    bog_t[:, blk, kt : kt + 1], mib_sb[:, :],
                                    Alu.add, Alu.subtract,
                                )

                            for kt in range(KT):
                                pending.append(
                                    lambda kt=kt, step=step: step(kt)
                                )

                    for nt in range(NTILES):
                        emit_mains(nt)
                    if not last:
                        wtmps_nxt = dequant(tiles_nxt, (li + 1) % NL)
                        tiles, wtmps = tiles_nxt, wtmps_nxt

                drip(len(pending))
                h_fin = hb[(inp0 + 5 + 1) % 3]
                for kt in range(KT):
                    nc.sync.dma_start(
                        y_d[:, kt, bass.ts(c, CHUNK)], h_fin[:, kt, :]
                    )

    nc.compile()
    return nc


def prep_inputs(x, wq, scales, bias, lora_a, lora_b, gamma, beta):
    """Host-side layout prep; returns per-core input maps."""
    wqc = wq.transpose(0, 2, 1).astype(np.int8) - 8  # [l, i, o] centered
    wqc = wqc.reshape(NL, KT, 128, DIM).transpose(0, 2, 1, 3).copy()  # [l,p,kt,o]

    G = scales.reshape(NL, DIM, DIM // GROUP)  # [l, o, gi]
    p_idx = np.arange(128)[:, None] // GROUP  # [128,1]
    kt_idx = np.arange(KT)[None, :] * (128 // GROUP)  # [1,8]
    gidx = p_idx + kt_idx  # [128, KT] group row index
    srep = G.transpose(0, 2, 1)[:, gidx, :].astype(F16).copy()  # [l,128,8,o]

    la_r = np.ascontiguousarray(lora_a.reshape(NL, RANK, KT, 128)).astype(F16)
    lb_r = np.ascontiguousarray(lora_b.transpose(0, 2, 1)).astype(F16)  # [l, r, o]

    bias_pp = bias.reshape(NL, KT, 128).transpose(2, 0, 1).astype(np.float32).copy()
    gamma_pp = gamma.reshape(5, KT, 128).transpose(2, 0, 1).astype(np.float32).copy()

    # beta/gamma (0 where gamma == 0), per-partition layout [128, 5, KT]
    gsafe = np.where(gamma == 0.0, 1.0, gamma)
    bog = np.where(gamma == 0.0, 0.0, beta / gsafe).astype(np.float32)  # [5, DIM]
    bog_pp = bog.reshape(5, KT, 128).transpose(2, 0, 1).astype(np.float32).copy()

    # diag(gamma) residual weights, partition-major: gdiag[b, p, kt, m] =
    # gamma[b, kt*128+p] if p == m else 0
    gdiag = np.zeros((5, 128, KT, 128), np.float32)
    idx = np.arange(128)
    gdiag[:, idx, :, idx] = gamma.reshape(5, KT, 128).transpose(2, 0, 1)
    gdiag = gdiag.astype(F16)

    shared = {
        "wqc": wqc, "srep": srep, "la_r": la_r, "lb_r": lb_r,
        "bias_pp": bias_pp, "gamma_pp": gamma_pp,
        "bog_pp": bog_pp, "gdiag": gdiag,
        "ones": np.ones((128, 128), F16),
        "ident": np.eye(128, dtype=F16),
    }
    in_maps = []
    for c in range(x.shape[0] // RPC):
        xs = x[c * RPC : (c + 1) * RPC]  # [rows, 1024]
        x_t = np.ascontiguousarray(
            xs.T.reshape(KT, 128, RPC).transpose(1, 0, 2)
        ).astype(F16)
        in_maps.append({"x_t": x_t, **shared})
    return in_maps


def unshard_output(results):
    outs = []
    for r in results:
        y_t = np.asarray(r["y_t"]).reshape(128, KT, RPC)
        outs.append(y_t.transpose(2, 1, 0).reshape(RPC, DIM))
    return np.ascontiguousarray(np.concatenate(outs, axis=0), dtype=np.float32)


def kernel(x, wq, scales, bias, lora_a, lora_b, gamma, beta):
    x, wq, scales, bias, lora_a, lora_b, gamma, beta = (
        np.asarray(a) for a in (x, wq, scales, bias, lora_a, lora_b, gamma, beta)
    )
    nc = build_kernel()
    in_maps = prep_inputs(x, wq, scales, bias, lora_a, lora_b, gamma, beta)
    res = run_bass_kernel_spmd(nc, in_maps, list(range(N_CORES)))
    return unshard_output(res.results)


# revision 29
# speedup vs baseline: 1.0535x; 1.0535x over previous
"""TRN2 Bass kernel for nn_CustomQLoRABigNet: 6 blocks x (3 QLoRA linears),
ReLU, residual, LayerNorm. Data-parallel over 8 NeuronCores (4096 rows each).

v4 design:
- All matmul operands f16 (PSUM f32). LoRA is folded into the dequantized
  weight on-chip: W' = dequant(q, s) + (lb @ la)^T, built with 16 small PE
  passes per layer; each layer is then one dense 1024x1024 matmul.
- chunk = 2048 columns per weight pass (2 chunks per core).
- 3-buffer hidden-state rotation; residuals are added in PSUM via
  identity / diag(gamma) matmuls (no vector-engine residual work).
- LayerNorm gamma/beta are folded away: gamma scales the next layer's
  weight during dequant (scalar_tensor_tensor, free) and the residual
  diag matmul; beta/gamma is carried inside the stored hidden state via
  a K=2 broadcast matmul ([ones; beta/gamma]^T @ [m*is; -1]). The LN
  apply is then just one mult + one sub per 128x512 tile on the DVE.
- LN stats via inline accumulating PE matmuls (lag-one-ot), inv-std via
  a raw Rsqrt activation (one act-table set -> no ACT_TABLE_LOAD thrash).
"""

import sys

sys.path.insert(0, "/opt/trn_rl_repo")

import numpy as np
import ml_dtypes

import concourse.bass as bass
from concourse import bacc, mybir
import concourse.tile as tile
from concourse.bass_utils import run_bass_kernel_spmd

f32 = mybir.dt.float32
f32r = mybir.dt.float32r
f16 = mybir.dt.float16
i8 = mybir.dt.int8
AF = mybir.ActivationFunctionType
Alu = mybir.AluOpType

N_CORES = 8
DIM = 1024
KT = 8  # 1024 / 128 partition tiles
NL = 18
RANK = 32
GROUP = 16
BATCH = 32768
RPC = BATCH // N_CORES  # rows per core
CHUNK = 2048  # columns (rows of x) processed per weight pass
NCH = RPC // CHUNK
NT = 512  # matmul moving free dim (one PSUM bank)
NTILES = CHUNK // NT
EPS = 1e-5
F16 = np.float16
USE_RSQRT = True

# layers whose input is a LayerNorm output (j0 of blocks 1..5): their
# weights absorb that LN's gamma
POST_LN = {3, 6, 9, 12, 15}


def build_kernel():
    nc = bacc.Bacc()

    x_d = nc.declare_dram_parameter("x_t", [128, KT, RPC], f16, False)
    wq_d = nc.declare_dram_parameter("wqc", [NL, 128, KT, DIM], i8, False)
    sr_d = nc.declare_dram_parameter("srep", [NL, 128, KT, DIM], f16, False)
    la_d = nc.declare_dram_parameter("la_r", [NL, RANK, KT, 128], f16, False)
    lb_d = nc.declare_dram_parameter("lb_r", [NL, RANK, DIM], f16, False)
    bi_d = nc.declare_dram_parameter("bias_pp", [128, NL, KT], f32, False)
    ga_d = nc.declare_dram_parameter("gamma_pp", [128, 5, KT], f32, False)
    bg_d = nc.declare_dram_parameter("bog_pp", [128, 5, KT], f32, False)
    gd_d = nc.declare_dram_parameter("gdiag", [5, 128, KT, 128], f16, False)
    on_d = nc.declare_dram_parameter("ones", [128, 128], f16, False)
    id_d = nc.declare_dram_parameter("ident", [128, 128], f16, False)
    y_d = nc.declare_dram_parameter("y_t", [128, KT, RPC], f16, True)

    with tile.TileContext(nc) as tc:
        with (
            tc.tile_pool(name="persist", bufs=1) as pp,
            tc.tile_pool(name="wts", bufs=2) as wp,
            tc.tile_pool(name="stage", bufs=1) as qp,
            tc.tile_pool(name="work", bufs=2) as sp,
            tc.tile_pool(name="ps", bufs=1, space="PSUM") as ps,
        ):
            h0 = pp.tile([128, KT, CHUNK], f16)
            h1 = pp.tile([128, KT, CHUNK], f16)
            h2 = pp.tile([128, KT, CHUNK], f16)
            hb = [h0, h1, h2]
            bias_t = pp.tile([128, NL, KT], f32)
            nc.sync.dma_start(bias_t[:, :, :], bi_d[:, :, :])
            gamma_t = pp.tile([128, 5, KT], f32)
            nc.sync.dma_start(gamma_t[:, :, :], ga_d[:, :, :])
            bog_t = pp.tile([128, 5, KT], f32)
            nc.sync.dma_start(bog_t[:, :, :], bg_d[:, :, :])
            ones_t = pp.tile([128, 128], f16)
            nc.sync.dma_start(ones_t[:, :], on_d[:, :])
            ident_t = pp.tile([128, 128], f16)
            nc.sync.dma_start(ident_t[:, :], id_d[:, :])
            ones_col = ones_t[:, 0:1]  # [128,1] stats lhsT
            ones_row = ones_t[0:1, :]  # [1,128] broadcast lhsT
            eps_t = pp.tile([1, 1], f32)
            nc.vector.memset(eps_t[:, :], EPS)

            def rsqrt(out, in_, bias_ap):
                # raw Rsqrt activation: bass's wrapper rejects it for
                # accuracy reasons irrelevant at this tolerance, and it
                # keeps the Scalar engine inside one act-func table set
                eng = nc.scalar
                ins = [
                    eng.lower_ap(in_),
                    eng.lower_ap(bias_ap),
                    mybir.ImmediateValue(dtype=mybir.dt.float32, value=1.0),
                    mybir.ImmediateValue(dtype=mybir.dt.float32, value=0.0),
                ]
                return eng.add_instruction(
                    mybir.InstActivation(
                        name=eng.bass.get_next_instruction_name(),
                        func=AF.Rsqrt,
                        ins=ins,
                        outs=[eng.lower_ap(out)],
                    )
                )

            def load_layer(li):
                blk, j = li // 3, li % 3
                wq_t = qp.tile([128, KT, DIM], i8, tag="wq")
                nc.sync.dma_start(wq_t[:, :, :], wq_d[li, :, :, :])
                sr_t = qp.tile([128, KT, DIM], f16, tag="sr")
                nc.sync.dma_start(sr_t[:, :, :], sr_d[li, :, :, :])
                la_t = wp.tile([RANK, KT, 128], f16, tag="la")
                nc.sync.dma_start(la_t[:, :, :], la_d[li, :, :, :])
                lb_t = wp.tile([RANK, DIM], f16, tag="lb")
                nc.sync.dma_start(lb_t[:, :], lb_d[li, :, :])
                gd_t = None
                if j == 2 and blk >= 1:
                    gd_t = wp.tile([128, KT, 128], f16, tag="gd")
                    nc.sync.dma_start(gd_t[:, :, :], gd_d[blk - 1, :, :, :])
                return wq_t, sr_t, la_t, lb_t, gd_t

            def dequant(tiles, li):
                # oh-major so the oh=0 half of w' is complete first
                wq_t, sr_t = tiles[0], tiles[1]
                wtmps = {}
                for oh in range(2):
                    for kt in range(KT):
                        oc = bass.ts(oh, NT)
                        wtmp = sp.tile([128, NT], f32, tag="wtmp", bufs=8)
                        if li in POST_LN:
                            g = gamma_t[:, li // 3 - 1, kt : kt + 1]
                            nc.vector.scalar_tensor_tensor(
                                wtmp[:, :], wq_t[:, kt, oc], g,
                                sr_t[:, kt, oc], Alu.mult, Alu.mult,
                            )
                        else:
                            nc.vector.tensor_mul(
                                wtmp[:, :], wq_t[:, kt, oc], sr_t[:, kt, oc]
                            )
                        wtmps[(kt, oh)] = wtmp
                return wtmps

            def build_w(tiles, wtmps, li):
                # W'[i, o] = gamma_i * ((q-8)*s + sum_r la[r,i]*lb[o,r])
                la_t, lb_t = tiles[2], tiles[3]
                w_t = wp.tile([128, KT, DIM], f16, tag="wt")
                for oh in range(2):
                    for kt in range(KT):
                        oc = bass.ts(oh, NT)
                        pb = ps.tile([128, NT], f32, tag="pb", bufs=2)
                        nc.tensor.matmul(
                            pb[:, :],
                            lhsT=la_t[:, kt, :],
                            rhs=lb_t[:, oc],
                            start=True,
                            stop=True,
                        )
                        if li in POST_LN:
                            g = gamma_t[:, li // 3 - 1, kt : kt + 1]
                            nc.vector.scalar_tensor_tensor(
                                w_t[:, kt, oc], pb[:, :], g,
                                wtmps[(kt, oh)][:, :], Alu.mult, Alu.add,
                            )
                        else:
                            nc.vector.tensor_add(
                                w_t[:, kt, oc], wtmps[(kt, oh)][:, :], pb[:, :]
                            )
                return w_t

            for c in range(NCH):
                inp0 = c % 3  # buffer receiving this chunk's x
                xbuf = hb[inp0]
                for kt in range(KT):
                    for nt in range(NTILES):
                        nc.sync.dma_start(
                            xbuf[:, kt, bass.ts(nt, NT)],
                            x_d[:, kt, bass.ts(c * NTILES + nt, NT)],
                        )

                if c == 0:
                    tiles = load_layer(0)
                    wtmps = dequant(tiles, 0)

                # LN finalization steps (one K=2 bcast matmul + DVE
                # mult/sub per kt) drip one per ot-chain across the
                # following tiles, so the PE stream is never gated by
                # the DVE apply ops (WAR on the rotating mib banks).
                pending = []

                def drip(n=1):
                    for _ in range(min(n, len(pending))):
                        pending.pop(0)()

                for li in range(NL):
                    blk, j = li // 3, li % 3
                    i = (inp0 + blk) % 3  # block input (residual) buffer
                    if j == 0:
                        src, dst = i, (i + 1) % 3
                    elif j == 1:
                        src, dst = (i + 1) % 3, (i + 2) % 3
                    else:
                        src, dst = (i + 2) % 3, (i + 1) % 3
                    h_in, h_out, r_buf = hb[src], hb[dst], hb[i]
                    ln_here = j == 2 and blk < 5
                    gd_t = tiles[4]

                    w_t = build_w(tiles, wtmps, li)
                    last = c == NCH - 1 and li == NL - 1
                    if not last:
                        tiles_nxt = load_layer((li + 1) % NL)

                    def emit_mains(nt):
                        cols = bass.ts(nt, NT)
                        sacc = []
                        s12 = None
                        if ln_here:
                            s12 = ps.tile([33, NT], f32, tag="s12", bufs=1)
                        for ot in range(KT):
                            y_ps = ps.tile([128, NT], f32, tag="y", bufs=2)
                            for kt in range(KT):
                                nc.tensor.matmul(
                                    y_ps[:, :],
                                    lhsT=w_t[:, kt, bass.ts(ot, 128)],
                                    rhs=h_in[:, kt, cols],
                                    start=(kt == 0),
                                    stop=(kt == KT - 1 and j != 2),
                                )
                            if j == 2:
                                # residual (diag(gamma) for blocks >= 1)
                                nc.tensor.matmul(
                                    y_ps[:, :],
                                    lhsT=ident_t[:, :] if blk == 0
                                    else gd_t[:, ot, :],
                                    rhs=r_buf[:, ot, cols],
                                    start=False,
                                    stop=True,
                                )
                            drip()
                            nc.scalar.activation(
                                h_out[:, ot, cols],
                                y_ps[:, :],
                                AF.Relu if j < 2 else AF.Identity,
                                bias=bias_t[:, li, ot : ot + 1],
                            )
                            if ln_here:
                                hsq = sp.tile([128, NT], f16, tag="hsq", bufs=3)
                                nc.scalar.square(hsq[:, :], h_out[:, ot, cols])
                                sacc.append((h_out[:, ot, cols], hsq))
                                # lag-one-ot stats so the PE never waits
                                # on the activation/square writes
                                if ot >= 1:
                                    ho_p, hq_p = sacc[ot - 1]
                                    nc.tensor.matmul(
                                        s12[0:1, :], lhsT=ones_col, rhs=ho_p,
                                        start=(ot == 1), stop=False,
                                    )
                                    nc.tensor.matmul(
                                        s12[32:33, :], lhsT=ones_col,
                                        rhs=hq_p[:, :],
                                        start=(ot == 1), stop=False,
                                    )
                        if ln_here:
                            ho_p, hq_p = sacc[KT - 1]
                            nc.tensor.matmul(
                                s12[0:1, :], lhsT=ones_col, rhs=ho_p,
                                start=False, stop=True,
                            )
                            nc.tensor.matmul(
                                s12[32:33, :], lhsT=ones_col, rhs=hq_p[:, :],
                                start=False, stop=True,
                            )
                            # inv-std chain (small ops, off the PE)
                            m_sb = sp.tile([1, NT], f32, tag="m", bufs=1)
                            nc.vector.tensor_scalar(
                                m_sb[:, :], s12[0:1, :], 1.0 / DIM, None,
                                Alu.mult,
                            )
                            msq = sp.tile([1, NT], f32, tag="msq", bufs=1)
                            nc.vector.tensor_mul(msq[:, :], m_sb[:, :], m_sb[:, :])
                            ve = sp.tile([1, NT], f32, tag="ve", bufs=1)
                            nc.vector.scalar_tensor_tensor(
                                ve[:, :], s12[32:33, :], 1.0 / DIM, msq[:, :],
                                Alu.mult, Alu.subtract,
                            )
                            is_sb = sp.tile([1, NT], f16, tag="isb", bufs=2)
                            if USE_RSQRT:
                                rsqrt(is_sb[:, :], ve[:, :], eps_t[:, :])
                            else:
                                lnv = sp.tile([1, NT], f32, tag="lnv", bufs=1)
                                nc.scalar.activation(
                                    lnv[:, :], ve[:, :], AF.Ln, bias=eps_t[:, :]
                                )
                                nc.scalar.activation(
                                    is_sb[:, :], lnv[:, :], AF.Exp, scale=-0.5
                                )
                            mis = sp.tile([1, NT], f16, tag="mis", bufs=2)
                            nc.vector.tensor_mul(
                                mis[:, :], m_sb[:, :], is_sb[:, :]
                            )

                            boxes = []

                            def step(kt, cols=cols, is_sb=is_sb, mis=mis,
                                     blk=blk, h_out=h_out, boxes=boxes):
                                if kt == 0:
                                    ib = ps.tile([128, NT], f32, tag="ib",
                                                 bufs=2)
                                    nc.tensor.matmul(
                                        ib[:, :], lhsT=ones_row,
                                        rhs=is_sb[:, :],
                                        start=True, stop=True,
                                    )
                                    mib = ps.tile([128, NT], f32, tag="mib",
                                                  bufs=1)
                                    nc.tensor.matmul(
                                        mib[:, :], lhsT=ones_row,
                                        rhs=mis[:, :],
                                        start=True, stop=True,
                                    )
                                    boxes.extend([ib, mib])
                                ib, mib = boxes
                                nc.vector.tensor_mul(
                                    h_out[:, kt, cols], h_out[:, kt, cols],
                                    ib[:, :],
                                )
                                # h = (h*ib + beta/gamma[p]) - m*is[n]
                                nc.vector.scalar_tensor_tensor(
                                    h_out[:, kt, cols], h_out[:, kt, cols],
                                    bog_t[:, blk, kt : kt + 1], mib[:, :],
                                    Alu.add, Alu.subtract,
                                )

                            for kt in range(KT):
                                pending.append(
                                    lambda kt=kt, step=step: step(kt)
                                )

                    for nt in range(NTILES):
                        emit_mains(nt)
                    if not last:
                        wtmps_nxt = dequant(tiles_nxt, (li + 1) % NL)
                        tiles, wtmps = tiles_nxt, wtmps_nxt

                drip(len(pending))
                h_fin = hb[(inp0 + 5 + 1) % 3]
                for kt in range(KT):
                    nc.sync.dma_start(
                        y_d[:, kt, bass.ts(c, CHUNK)], h_fin[:, kt, :]
                    )

    nc.compile()
    return nc


def prep_inputs(x, wq, scales, bias, lora_a, lora_b, gamma, beta):
    """Host-side layout prep; returns per-core input maps."""
    wqc = wq.transpose(0, 2, 1).astype(np.int8) - 8  # [l, i, o] centered
    wqc = wqc.reshape(NL, KT, 128, DIM).transpose(0, 2, 1, 3).copy()  # [l,p,kt,o]

    G = scales.reshape(NL, DIM, DIM // GROUP)  # [l, o, gi]
    p_idx = np.arange(128)[:, None] // GROUP  # [128,1]
    kt_idx = np.arange(KT)[None, :] * (128 // GROUP)  # [1,8]
    gidx = p_idx + kt_idx  # [128, KT] group row index
    srep = G.transpose(0, 2, 1)[:, gidx, :].astype(F16).copy()  # [l,128,8,o]

    la_r = np.ascontiguousarray(lora_a.reshape(NL, RANK, KT, 128)).astype(F16)
    lb_r = np.ascontiguousarray(lora_b.transpose(0, 2, 1)).astype(F16)  # [l, r, o]

    bias_pp = bias.reshape(NL, KT, 128).transpose(2, 0, 1).astype(np.float32).copy()
    gamma_pp = gamma.reshape(5, KT, 128).transpose(2, 0, 1).astype(np.float32).copy()

    # beta/gamma (0 where gamma == 0), per-partition layout [128, 5, KT]
    gsafe = np.where(gamma == 0.0, 1.0, gamma)
    bog = np.where(gamma == 0.0, 0.0, beta / gsafe).astype(np.float32)  # [5, DIM]
    bog_pp = bog.reshape(5, KT, 128).transpose(2, 0, 1).astype(np.float32).copy()

    # diag(gamma) residual weights, partition-major: gdiag[b, p, kt, m] =
    # gamma[b, kt*128+p] if p == m else 0
    gdiag = np.zeros((5, 128, KT, 128), np.float32)
    idx = np.arange(128)
    gdiag[:, idx, :, idx] = gamma.reshape(5, KT, 128).transpose(2, 0, 1)
    gdiag = gdiag.astype(F16)

    shared = {
        "wqc": wqc, "srep": srep, "la_r": la_r, "lb_r": lb_r,
        "bias_pp": bias_pp, "gamma_pp": gamma_pp,
        "bog_pp": bog_pp, "gdiag": gdiag,
        "ones": np.ones((128, 128), F16),
        "ident": np.eye(128, dtype=F16),
    }
    in_maps = []
    for c in range(x.shape[0] // RPC):
        xs = x[c * RPC : (c + 1) * RPC]  # [rows, 1024]
        x_t = np.ascontiguousarray(
            xs.T.reshape(KT, 128, RPC).transpose(1, 0, 2)
        ).astype(F16)
        in_maps.append({"x_t": x_t, **shared})
    return in_maps


def unshard_output(results):
    outs = []
    for r in results:
        y_t = np.asarray(r["y_t"]).reshape(128, KT, RPC)
        outs.append(y_t.transpose(2, 1, 0).reshape(RPC, DIM))
    return np.ascontiguousarray(np.concatenate(outs, axis=0), dtype=np.float32)


def kernel(x, wq, scales, bias, lora_a, lora_b, gamma, beta):
    x, wq, scales, bias, lora_a, lora_b, gamma, beta = (
        np.asarray(a) for a in (x, wq, scales, bias, lora_a, lora_b, gamma, beta)
    )
    nc = build_kernel()
    in_maps = prep_inputs(x, wq, scales, bias, lora_a, lora_b, gamma, beta)
    res = run_bass_kernel_spmd(nc, in_maps, list(range(N_CORES)))
    return unshard_output(res.results)


# revision 32
# speedup vs baseline: 1.1675x; 1.1083x over previous
"""TRN2 Bass kernel for nn_CustomQLoRABigNet: 6 blocks x (3 QLoRA linears),
ReLU, residual, LayerNorm. Data-parallel over 8 NeuronCores (4096 rows each).

v4 design:
- All matmul operands f16 (PSUM f32). LoRA is folded into the dequantized
  weight on-chip: W' = dequant(q, s) + (lb @ la)^T, built with 16 small PE
  passes per layer; each layer is then one dense 1024x1024 matmul.
- chunk = 2048 columns per weight pass (2 chunks per core).
- 3-buffer hidden-state rotation; residuals are added in PSUM via
  identity / diag(gamma) matmuls (no vector-engine residual work).
- LayerNorm gamma/beta are folded away: gamma scales the next layer's
  weight during dequant (scalar_tensor_tensor, free) and the residual
  diag matmul; beta/gamma is carried inside the stored hidden state via
  a K=2 broadcast matmul ([ones; beta/gamma]^T @ [m*is; -1]). The LN
  apply is then just one mult + one sub per 128x512 tile on the DVE.
- LN stats via inline accumulating PE matmuls (lag-one-ot), inv-std via
  a raw Rsqrt activation (one act-table set -> no ACT_TABLE_LOAD thrash).
"""

import sys

sys.path.insert(0, "/opt/trn_rl_repo")

import numpy as np
import ml_dtypes

import concourse.bass as bass
from concourse import bacc, mybir
import concourse.tile as tile
from concourse.bass_utils import run_bass_kernel_spmd

f32 = mybir.dt.float32
f32r = mybir.dt.float32r
f16 = mybir.dt.float16
i8 = mybir.dt.int8
AF = mybir.ActivationFunctionType
Alu = mybir.AluOpType

N_CORES = 8
DIM = 1024
KT = 8  # 1024 / 128 partition tiles
NL = 18
RANK = 32
GROUP = 16
BATCH = 32768
RPC = BATCH // N_CORES  # rows per core
CHUNK = 2048  # columns (rows of x) processed per weight pass
NCH = RPC // CHUNK
NT = 512  # matmul moving free dim (one PSUM bank)
NTILES = CHUNK // NT
EPS = 1e-5
F16 = np.float16
USE_RSQRT = True

# layers whose input is a LayerNorm output (j0 of blocks 1..5): their
# weights absorb that LN's gamma
POST_LN = {3, 6, 9, 12, 15}


def build_kernel():
    nc = bacc.Bacc()

    x_d = nc.declare_dram_parameter("x_t", [128, KT, RPC], f16, False)
    wq_d = nc.declare_dram_parameter("wqc", [NL, 128, KT, DIM], i8, False)
    sr_d = nc.declare_dram_parameter("srep", [NL, 128, KT, DIM], f16, False)
    la_d = nc.declare_dram_parameter("la_r", [NL, RANK, KT, 128], f16, False)
    lb_d = nc.declare_dram_parameter("lb_r", [NL, RANK, DIM], f16, False)
    bi_d = nc.declare_dram_parameter("bias_pp", [128, NL, KT], f32, False)
    ga_d = nc.declare_dram_parameter("gamma_pp", [128, 5, KT], f32, False)
    bg_d = nc.declare_dram_parameter("bog_pp", [128, 5, KT], f32, False)
    gd_d = nc.declare_dram_parameter("gdiag", [5, 128, KT, 128], f16, False)
    on_d = nc.declare_dram_parameter("ones", [128, 128], f16, False)
    id_d = nc.declare_dram_parameter("ident", [128, 128], f16, False)
    w2_d = nc.declare_dram_parameter("w2", [NL, 128, KT, DIM], f16, True)
    y_d = nc.declare_dram_parameter("y_t", [128, KT, RPC], f16, True)

    with tile.TileContext(nc) as tc:
        with (
            tc.tile_pool(name="persist", bufs=1) as pp,
            tc.tile_pool(name="wts", bufs=2) as wp,
            tc.tile_pool(name="stage", bufs=1) as qp,
            tc.tile_pool(name="work", bufs=2) as sp,
            tc.tile_pool(name="ps", bufs=1, space="PSUM") as ps,
        ):
            h0 = pp.tile([128, KT, CHUNK], f16)
            h1 = pp.tile([128, KT, CHUNK], f16)
            h2 = pp.tile([128, KT, CHUNK], f16)
            hb = [h0, h1, h2]
            bias_t = pp.tile([128, NL, KT], f32)
            nc.sync.dma_start(bias_t[:, :, :], bi_d[:, :, :])
            gamma_t = pp.tile([128, 5, KT], f32)
            nc.sync.dma_start(gamma_t[:, :, :], ga_d[:, :, :])
            bog_t = pp.tile([128, 5, KT], f32)
            nc.sync.dma_start(bog_t[:, :, :], bg_d[:, :, :])
            ones_t = pp.tile([128, 128], f16)
            nc.sync.dma_start(ones_t[:, :], on_d[:, :])
            ident_t = pp.tile([128, 128], f16)
            nc.sync.dma_start(ident_t[:, :], id_d[:, :])
            ones_col = ones_t[:, 0:1]  # [128,1] stats lhsT
            ones_row = ones_t[0:1, :]  # [1,128] broadcast lhsT
            eps_t = pp.tile([1, 1], f32)
            nc.vector.memset(eps_t[:, :], EPS)

            def rsqrt(out, in_, bias_ap):
                # raw Rsqrt activation: bass's wrapper rejects it for
                # accuracy reasons irrelevant at this tolerance, and it
                # keeps the Scalar engine inside one act-func table set
                eng = nc.scalar
                ins = [
                    eng.lower_ap(in_),
                    eng.lower_ap(bias_ap),
                    mybir.ImmediateValue(dtype=mybir.dt.float32, value=1.0),
                    mybir.ImmediateValue(dtype=mybir.dt.float32, value=0.0),
                ]
                return eng.add_instruction(
                    mybir.InstActivation(
                        name=eng.bass.get_next_instruction_name(),
                        func=AF.Rsqrt,
                        ins=ins,
                        outs=[eng.lower_ap(out)],
                    )
                )

            def load_layer(li):
                blk, j = li // 3, li % 3
                wq_t = qp.tile([128, KT, DIM], i8, tag="wq")
                nc.sync.dma_start(wq_t[:, :, :], wq_d[li, :, :, :])
                sr_t = qp.tile([128, KT, DIM], f16, tag="sr")
                nc.sync.dma_start(sr_t[:, :, :], sr_d[li, :, :, :])
                la_t = wp.tile([RANK, KT, 128], f16, tag="la")
                nc.sync.dma_start(la_t[:, :, :], la_d[li, :, :, :])
                lb_t = wp.tile([RANK, DIM], f16, tag="lb")
                nc.sync.dma_start(lb_t[:, :], lb_d[li, :, :])
                gd_t = None
                if j == 2 and blk >= 1:
                    gd_t = wp.tile([128, KT, 128], f16, tag="gd")
                    nc.sync.dma_start(gd_t[:, :, :], gd_d[blk - 1, :, :, :])
                return wq_t, sr_t, la_t, lb_t, gd_t

            def dequant(tiles, li):
                # oh-major so the oh=0 half of w' is complete first
                wq_t, sr_t = tiles[0], tiles[1]
                wtmps = {}
                for oh in range(2):
                    for kt in range(KT):
                        oc = bass.ts(oh, NT)
                        wtmp = sp.tile([128, NT], f32, tag="wtmp", bufs=8)
                        if li in POST_LN:
                            g = gamma_t[:, li // 3 - 1, kt : kt + 1]
                            nc.vector.scalar_tensor_tensor(
                                wtmp[:, :], wq_t[:, kt, oc], g,
                                sr_t[:, kt, oc], Alu.mult, Alu.mult,
                            )
                        else:
                            nc.vector.tensor_mul(
                                wtmp[:, :], wq_t[:, kt, oc], sr_t[:, kt, oc]
                            )
                        wtmps[(kt, oh)] = wtmp
                return wtmps

            def build_w(tiles, wtmps, li):
                # W'[i, o] = gamma_i * ((q-8)*s + sum_r la[r,i]*lb[o,r])
                la_t, lb_t = tiles[2], tiles[3]
                w_t = wp.tile([128, KT, DIM], f16, tag="wt")
                for oh in range(2):
                    for kt in range(KT):
                        oc = bass.ts(oh, NT)
                        pb = ps.tile([128, NT], f32, tag="pb", bufs=2)
                        nc.tensor.matmul(
                            pb[:, :],
                            lhsT=la_t[:, kt, :],
                            rhs=lb_t[:, oc],
                            start=True,
                            stop=True,
                        )
                        if li in POST_LN:
                            g = gamma_t[:, li // 3 - 1, kt : kt + 1]
                            nc.vector.scalar_tensor_tensor(
                                w_t[:, kt, oc], pb[:, :], g,
                                wtmps[(kt, oh)][:, :], Alu.mult, Alu.add,
                            )
                        else:
                            nc.vector.tensor_add(
                                w_t[:, kt, oc], wtmps[(kt, oh)][:, :], pb[:, :]
                            )
                return w_t

            for c in range(NCH):
                inp0 = c % 3  # buffer receiving this chunk's x
                xbuf = hb[inp0]
                for kt in range(KT):
                    for nt in range(NTILES):
                        nc.sync.dma_start(
                            xbuf[:, kt, bass.ts(nt, NT)],
                            x_d[:, kt, bass.ts(c * NTILES + nt, NT)],
                        )

                if c == 0:
                    tiles = load_layer(0)
                    wtmps = dequant(tiles, 0)
                else:
                    # chunk 1 reloads the spilled W' instead of rebuilding
                    w_nxt = wp.tile([128, KT, DIM], f16, tag="wt", name="w_pre")
                    nc.sync.dma_start(w_nxt[:, :, :], w2_d[0, :, :, :])
                    gd_nxt = None

                # LN finalization steps (one K=2 bcast matmul + DVE
                # mult/sub per kt) drip one per ot-chain across the
                # following tiles, so the PE stream is never gated by
                # the DVE apply ops (WAR on the rotating mib banks).
                pending = []

                def drip(n=1):
                    for _ in range(min(n, len(pending))):
                        pending.pop(0)()

                for li in range(NL):
                    blk, j = li // 3, li % 3
                    i = (inp0 + blk) % 3  # block input (residual) buffer
                    if j == 0:
                        src, dst = i, (i + 1) % 3
                    elif j == 1:
                        src, dst = (i + 1) % 3, (i + 2) % 3
                    else:
                        src, dst = (i + 2) % 3, (i + 1) % 3
                    h_in, h_out, r_buf = hb[src], hb[dst], hb[i]
                    ln_here = j == 2 and blk < 5
                    if c == 0:
                        gd_t = tiles[4]
                        w_t = build_w(tiles, wtmps, li)
                        nc.sync.dma_start(w2_d[li, :, :, :], w_t[:, :, :])
                        if li < NL - 1:
                            tiles_nxt = load_layer(li + 1)
                    else:
                        w_t, gd_t = w_nxt, gd_nxt
                        if li < NL - 1:
                            w_nxt = wp.tile([128, KT, DIM], f16, tag="wt",
                                            name="w_pre")
                            nc.sync.dma_start(
                                w_nxt[:, :, :], w2_d[li + 1, :, :, :]
                            )
                            gd_nxt = None
                            if (li + 1) % 3 == 2 and (li + 1) // 3 >= 1:
                                gd_nxt = wp.tile([128, KT, 128], f16,
                                                 tag="gd", name="gd_pre")
                                nc.sync.dma_start(
                                    gd_nxt[:, :, :],
                                    gd_d[(li + 1) // 3 - 1, :, :, :],
                                )

                    def emit_mains(nt):
                        cols = bass.ts(nt, NT)
                        sacc = []
                        s12 = None
                        if ln_here:
                            s12 = ps.tile([33, NT], f32, tag="s12", bufs=1)
                        for ot in range(KT):
                            y_ps = ps.tile([128, NT], f32, tag="y", bufs=2)
                            for kt in range(KT):
                                nc.tensor.matmul(
                                    y_ps[:, :],
                                    lhsT=w_t[:, kt, bass.ts(ot, 128)],
                                    rhs=h_in[:, kt, cols],
                                    start=(kt == 0),
                                    stop=(kt == KT - 1 and j != 2),
                                )
                            if j == 2:
                                # residual (diag(gamma) for blocks >= 1)
                                nc.tensor.matmul(
                                    y_ps[:, :],
                                    lhsT=ident_t[:, :] if blk == 0
                                    else gd_t[:, ot, :],
                                    rhs=r_buf[:, ot, cols],
                                    start=False,
                                    stop=True,
                                )
                            drip()
                            nc.scalar.activation(
                                h_out[:, ot, cols],
                                y_ps[:, :],
                                AF.Relu if j < 2 else AF.Identity,
                                bias=bias_t[:, li, ot : ot + 1],
                            )
                            if ln_here:
                                hsq = sp.tile([128, NT], f16, tag="hsq", bufs=3)
                                nc.scalar.square(hsq[:, :], h_out[:, ot, cols])
                                sacc.append((h_out[:, ot, cols], hsq))
                                # lag-one-ot stats so the PE never waits
                                # on the activation/square writes
                                if ot >= 1:
                                    ho_p, hq_p = sacc[ot - 1]
                                    nc.tensor.matmul(
                                        s12[0:1, :], lhsT=ones_col, rhs=ho_p,
                                        start=(ot == 1), stop=False,
                                    )
                                    nc.tensor.matmul(
                                        s12[32:33, :], lhsT=ones_col,
                                        rhs=hq_p[:, :],
                                        start=(ot == 1), stop=False,
                                    )
                        if ln_here:
                            ho_p, hq_p = sacc[KT - 1]
                            nc.tensor.matmul(
                                s12[0:1, :], lhsT=ones_col, rhs=ho_p,
                                start=False, stop=True,
                            )
                            nc.tensor.matmul(
                                s12[32:33, :], lhsT=ones_col, rhs=hq_p[:, :],
                                start=False, stop=True,
                            )
                            # inv-std chain (small ops, off the PE)
                            m_sb = sp.tile([1, NT], f32, tag="m", bufs=1)
                            nc.vector.tensor_scalar(
                                m_sb[:, :], s12[0:1, :], 1.0 / DIM, None,
                                Alu.mult,
                            )
                            msq = sp.tile([1, NT], f32, tag="msq", bufs=1)
                            nc.vector.tensor_mul(msq[:, :], m_sb[:, :], m_sb[:, :])
                            ve = sp.tile([1, NT], f32, tag="ve", bufs=1)
                            nc.vector.scalar_tensor_tensor(
                                ve[:, :], s12[32:33, :], 1.0 / DIM, msq[:, :],
                                Alu.mult, Alu.subtract,
                            )
                            is_sb = sp.tile([1, NT], f16, tag="isb", bufs=2)
                            if USE_RSQRT:
                                rsqrt(is_sb[:, :], ve[:, :], eps_t[:, :])
                            else:
                                lnv = sp.tile([1, NT], f32, tag="lnv", bufs=1)
                                nc.scalar.activation(
                                    lnv[:, :], ve[:, :], AF.Ln, bias=eps_t[:, :]
                                )
                                nc.scalar.activation(
                                    is_sb[:, :], lnv[:, :], AF.Exp, scale=-0.5
                                )
                            mis = sp.tile([1, NT], f16, tag="mis", bufs=2)
                            nc.vector.tensor_mul(
                                mis[:, :], m_sb[:, :], is_sb[:, :]
                            )

                            boxes = []

                            def step(kt, cols=cols, is_sb=is_sb, mis=mis,
                                     blk=blk, h_out=h_out, boxes=boxes):
                                if kt == 0:
                                    ib = ps.tile([128, NT], f32, tag="ib",
                                                 bufs=2)
                                    nc.tensor.matmul(
                                        ib[:, :], lhsT=ones_row,
                                        rhs=is_sb[:, :],
                                        start=True, stop=True,
                                    )
                                    mib = ps.tile([128, NT], f32, tag="mib",
                                                  bufs=1)
                                    nc.tensor.matmul(
                                        mib[:, :], lhsT=ones_row,
                                        rhs=mis[:, :],
                                        start=True, stop=True,
                                    )
                                    boxes.extend([ib, mib])
                                ib, mib = boxes
                                nc.vector.tensor_mul(
                                    h_out[:, kt, cols], h_out[:, kt, cols],
                                    ib[:, :],
                                )
                                # h = (h*ib + beta/gamma[p]) - m*is[n]
                                nc.vector.scalar_tensor_tensor(
                                    h_out[:, kt, cols], h_out[:, kt, cols],
                                    bog_t[:, blk, kt : kt + 1], mib[:, :],
                                    Alu.add, Alu.subtract,
                                )

                            for kt in range(KT):
                                pending.append(
                                    lambda kt=kt, step=step: step(kt)
                                )

                    for nt in range(NTILES):
                        emit_mains(nt)
                    if c == 0 and li < NL - 1:
                        wtmps = dequant(tiles_nxt, li + 1)
                        tiles = tiles_nxt

                drip(len(pending))
                h_fin = hb[(inp0 + 5 + 1) % 3]
                for kt in range(KT):
                    nc.sync.dma_start(
                        y_d[:, kt, bass.ts(c, CHUNK)], h_fin[:, kt, :]
                    )

    nc.compile()
    return nc


def prep_inputs(x, wq, scales, bias, lora_a, lora_b, gamma, beta):
    """Host-side layout prep; returns per-core input maps."""
    wqc = wq.transpose(0, 2, 1).astype(np.int8) - 8  # [l, i, o] centered
    wqc = wqc.reshape(NL, KT, 128, DIM).transpose(0, 2, 1, 3).copy()  # [l,p,kt,o]

    G = scales.reshape(NL, DIM, DIM // GROUP)  # [l, o, gi]
    p_idx = np.arange(128)[:, None] // GROUP  # [128,1]
    kt_idx = np.arange(KT)[None, :] * (128 // GROUP)  # [1,8]
    gidx = p_idx + kt_idx  # [128, KT] group row index
    srep = G.transpose(0, 2, 1)[:, gidx, :].astype(F16).copy()  # [l,128,8,o]

    la_r = np.ascontiguousarray(lora_a.reshape(NL, RANK, KT, 128)).astype(F16)
    lb_r = np.ascontiguousarray(lora_b.transpose(0, 2, 1)).astype(F16)  # [l, r, o]

    bias_pp = bias.reshape(NL, KT, 128).transpose(2, 0, 1).astype(np.float32).copy()
    gamma_pp = gamma.reshape(5, KT, 128).transpose(2, 0, 1).astype(np.float32).copy()

    # beta/gamma (0 where gamma == 0), per-partition layout [128, 5, KT]
    gsafe = np.where(gamma == 0.0, 1.0, gamma)
    bog = np.where(gamma == 0.0, 0.0, beta / gsafe).astype(np.float32)  # [5, DIM]
    bog_pp = bog.reshape(5, KT, 128).transpose(2, 0, 1).astype(np.float32).copy()

    # diag(gamma) residual weights, partition-major: gdiag[b, p, kt, m] =
    # gamma[b, kt*128+p] if p == m else 0
    gdiag = np.zeros((5, 128, KT, 128), np.float32)
    idx = np.arange(128)
    gdiag[:, idx, :, idx] = gamma.reshape(5, KT, 128).transpose(2, 0, 1)
    gdiag = gdiag.astype(F16)

    shared = {
        "wqc": wqc, "srep": srep, "la_r": la_r, "lb_r": lb_r,
        "bias_pp": bias_pp, "gamma_pp": gamma_pp,
        "bog_pp": bog_pp, "gdiag": gdiag,
        "ones": np.ones((128, 128), F16),
        "ident": np.eye(128, dtype=F16),
    }
    in_maps = []
    for c in range(x.shape[0] // RPC):
        xs = x[c * RPC : (c + 1) * RPC]  # [rows, 1024]
        x_t = np.ascontiguousarray(
            xs.T.reshape(KT, 128, RPC).transpose(1, 0, 2)
        ).astype(F16)
        in_maps.append({"x_t": x_t, **shared})
    return in_maps


def unshard_output(results):
    outs = []
    for r in results:
        y_t = np.asarray(r["y_t"]).reshape(128, KT, RPC)
        outs.append(y_t.transpose(2, 1, 0).reshape(RPC, DIM))
    return np.ascontiguousarray(np.concatenate(outs, axis=0), dtype=np.float32)


def kernel(x, wq, scales, bias, lora_a, lora_b, gamma, beta):
    x, wq, scales, bias, lora_a, lora_b, gamma, beta = (
        np.asarray(a) for a in (x, wq, scales, bias, lora_a, lora_b, gamma, beta)
    )
    nc = build_kernel()
    in_maps = prep_inputs(x, wq, scales, bias, lora_a, lora_b, gamma, beta)
    res = run_bass_kernel_spmd(nc, in_maps, list(range(N_CORES)))
    return unshard_output(res.results)


# revision 33
# speedup vs baseline: 1.2139x; 1.0397x over previous
"""TRN2 Bass kernel for nn_CustomQLoRABigNet: 6 blocks x (3 QLoRA linears),
ReLU, residual, LayerNorm. Data-parallel over 8 NeuronCores (4096 rows each).

v4 design:
- All matmul operands f16 (PSUM f32). LoRA is folded into the dequantized
  weight on-chip: W' = dequant(q, s) + (lb @ la)^T, built with 16 small PE
  passes per layer; each layer is then one dense 1024x1024 matmul.
- chunk = 2048 columns per weight pass (2 chunks per core).
- 3-buffer hidden-state rotation; residuals are added in PSUM via
  identity / diag(gamma) matmuls (no vector-engine residual work).
- LayerNorm gamma/beta are folded away: gamma scales the next layer's
  weight during dequant (scalar_tensor_tensor, free) and the residual
  diag matmul; beta/gamma is carried inside the stored hidden state via
  a K=2 broadcast matmul ([ones; beta/gamma]^T @ [m*is; -1]). The LN
  apply is then just one mult + one sub per 128x512 tile on the DVE.
- LN stats via inline accumulating PE matmuls (lag-one-ot), inv-std via
  a raw Rsqrt activation (one act-table set -> no ACT_TABLE_LOAD thrash).
"""

import sys

sys.path.insert(0, "/opt/trn_rl_repo")

import numpy as np
import ml_dtypes

import concourse.bass as bass
from concourse import bacc, mybir
import concourse.tile as tile
from concourse.bass_utils import run_bass_kernel_spmd

f32 = mybir.dt.float32
f32r = mybir.dt.float32r
f16 = mybir.dt.float16
i8 = mybir.dt.int8
AF = mybir.ActivationFunctionType
Alu = mybir.AluOpType

N_CORES = 8
DIM = 1024
KT = 8  # 1024 / 128 partition tiles
NL = 18
RANK = 32
GROUP = 16
BATCH = 32768
RPC = BATCH // N_CORES  # rows per core
CHUNK = 2048  # columns (rows of x) processed per weight pass
NCH = RPC // CHUNK
NT = 512  # matmul moving free dim (one PSUM bank)
NTILES = CHUNK // NT
EPS = 1e-5
F16 = np.float16
USE_RSQRT = True

# layers whose input is a LayerNorm output (j0 of blocks 1..5): their
# weights absorb that LN's gamma
POST_LN = {3, 6, 9, 12, 15}


def build_kernel():
    nc = bacc.Bacc()

    x_d = nc.declare_dram_parameter("x_t", [128, KT, RPC], f16, False)
    wq_d = nc.declare_dram_parameter("wqc", [NL, 128, KT, DIM], i8, False)
    sr_d = nc.declare_dram_parameter("srep", [NL, 128, KT, DIM], f16, False)
    la_d = nc.declare_dram_parameter("la_r", [NL, RANK, KT, 128], f16, False)
    lb_d = nc.declare_dram_parameter("lb_r", [NL, RANK, DIM], f16, False)
    bi_d = nc.declare_dram_parameter("bias_pp", [128, NL, KT], f32, False)
    ga_d = nc.declare_dram_parameter("gamma_pp", [128, 5, KT], f32, False)
    bg_d = nc.declare_dram_parameter("bog_pp", [128, 5, KT], f32, False)
    gd_d = nc.declare_dram_parameter("gdiag", [5, 128, KT, 128], f16, False)
    on_d = nc.declare_dram_parameter("ones", [128, 128], f16, False)
    id_d = nc.declare_dram_parameter("ident", [128, 128], f16, False)
    w2_d = nc.declare_dram_parameter("w2", [NL, 128, KT, DIM], f16, True)
    y_d = nc.declare_dram_parameter("y_t", [128, KT, RPC], f16, True)

    with tile.TileContext(nc) as tc:
        with (
            tc.tile_pool(name="persist", bufs=1) as pp,
            tc.tile_pool(name="wts", bufs=2) as wp,
            tc.tile_pool(name="stage", bufs=1) as qp,
            tc.tile_pool(name="work", bufs=2) as sp,
            tc.tile_pool(name="ps", bufs=1, space="PSUM") as ps,
        ):
            h0 = pp.tile([128, KT, CHUNK], f16)
            h1 = pp.tile([128, KT, CHUNK], f16)
            h2 = pp.tile([128, KT, CHUNK], f16)
            hb = [h0, h1, h2]
            bias_t = pp.tile([128, NL, KT], f32)
            nc.sync.dma_start(bias_t[:, :, :], bi_d[:, :, :])
            gamma_t = pp.tile([128, 5, KT], f32)
            nc.sync.dma_start(gamma_t[:, :, :], ga_d[:, :, :])
            bog_t = pp.tile([128, 5, KT], f32)
            nc.sync.dma_start(bog_t[:, :, :], bg_d[:, :, :])
            ones_t = pp.tile([128, 128], f16)
            nc.sync.dma_start(ones_t[:, :], on_d[:, :])
            ident_t = pp.tile([128, 128], f16)
            nc.sync.dma_start(ident_t[:, :], id_d[:, :])
            ones_col = ones_t[:, 0:1]  # [128,1] stats lhsT
            ones_row = ones_t[0:1, :]  # [1,128] broadcast lhsT
            eps_t = pp.tile([1, 1], f32)
            nc.vector.memset(eps_t[:, :], EPS)

            def rsqrt(out, in_, bias_ap):
                # raw Rsqrt activation: bass's wrapper rejects it for
                # accuracy reasons irrelevant at this tolerance, and it
                # keeps the Scalar engine inside one act-func table set
                eng = nc.scalar
                ins = [
                    eng.lower_ap(in_),
                    eng.lower_ap(bias_ap),
                    mybir.ImmediateValue(dtype=mybir.dt.float32, value=1.0),
                    mybir.ImmediateValue(dtype=mybir.dt.float32, value=0.0),
                ]
                return eng.add_instruction(
                    mybir.InstActivation(
                        name=eng.bass.get_next_instruction_name(),
                        func=AF.Rsqrt,
                        ins=ins,
                        outs=[eng.lower_ap(out)],
                    )
                )

            def load_layer(li):
                blk, j = li // 3, li % 3
                wq_t = qp.tile([128, KT, DIM], i8, tag="wq")
                nc.sync.dma_start(wq_t[:, :, :], wq_d[li, :, :, :])
                sr_t = qp.tile([128, KT, DIM], f16, tag="sr")
                nc.sync.dma_start(sr_t[:, :, :], sr_d[li, :, :, :])
                la_t = wp.tile([RANK, KT, 128], f16, tag="la")
                nc.sync.dma_start(la_t[:, :, :], la_d[li, :, :, :])
                lb_t = wp.tile([RANK, DIM], f16, tag="lb")
                nc.sync.dma_start(lb_t[:, :], lb_d[li, :, :])
                gd_t = None
                if j == 2 and blk >= 1:
                    gd_t = wp.tile([128, KT, 128], f16, tag="gd")
                    nc.sync.dma_start(gd_t[:, :, :], gd_d[blk - 1, :, :, :])
                return wq_t, sr_t, la_t, lb_t, gd_t

            def dequant(tiles, li):
                # oh-major so the oh=0 half of w' is complete first
                wq_t, sr_t = tiles[0], tiles[1]
                wtmps = {}
                for oh in range(2):
                    for kt in range(KT):
                        oc = bass.ts(oh, NT)
                        wtmp = sp.tile([128, NT], f32, tag="wtmp", bufs=8)
                        if li in POST_LN:
                            g = gamma_t[:, li // 3 - 1, kt : kt + 1]
                            nc.vector.scalar_tensor_tensor(
                                wtmp[:, :], wq_t[:, kt, oc], g,
                                sr_t[:, kt, oc], Alu.mult, Alu.mult,
                            )
                        else:
                            nc.vector.tensor_mul(
                                wtmp[:, :], wq_t[:, kt, oc], sr_t[:, kt, oc]
                            )
                        wtmps[(kt, oh)] = wtmp
                return wtmps

            def build_w(tiles, wtmps, li):
                # W'[i, o] = gamma_i * ((q-8)*s + sum_r la[r,i]*lb[o,r])
                la_t, lb_t = tiles[2], tiles[3]
                w_t = wp.tile([128, KT, DIM], f16, tag="wt")
                for oh in range(2):
                    for kt in range(KT):
                        oc = bass.ts(oh, NT)
                        pb = ps.tile([128, NT], f32, tag="pb", bufs=2)
                        nc.tensor.matmul(
                            pb[:, :],
                            lhsT=la_t[:, kt, :],
                            rhs=lb_t[:, oc],
                            start=True,
                            stop=True,
                        )
                        if li in POST_LN:
                            g = gamma_t[:, li // 3 - 1, kt : kt + 1]
                            nc.vector.scalar_tensor_tensor(
                                w_t[:, kt, oc], pb[:, :], g,
                                wtmps[(kt, oh)][:, :], Alu.mult, Alu.add,
                            )
                        else:
                            nc.vector.tensor_add(
                                w_t[:, kt, oc], wtmps[(kt, oh)][:, :], pb[:, :]
                            )
                return w_t

            for c in range(NCH):
                inp0 = c % 3  # buffer receiving this chunk's x
                xbuf = hb[inp0]
                for kt in range(KT):
                    for nt in range(NTILES):
                        nc.sync.dma_start(
                            xbuf[:, kt, bass.ts(nt, NT)],
                            x_d[:, kt, bass.ts(c * NTILES + nt, NT)],
                        )

                if c == 0:
                    tiles = load_layer(0)
                    wtmps = dequant(tiles, 0)
                else:
                    # chunk 1 reloads the spilled W' instead of rebuilding
                    w_nxt = wp.tile([128, KT, DIM], f16, tag="wt", name="w_pre")
                    nc.sync.dma_start(w_nxt[:, :, :], w2_d[0, :, :, :])
                    gd_nxt = None

                # LN finalization steps (one K=2 bcast matmul + DVE
                # mult/sub per kt) drip one per ot-chain across the
                # following tiles, so the PE stream is never gated by
                # the DVE apply ops (WAR on the rotating mib banks).
                pending = []

                def drip(n=1):
                    for _ in range(min(n, len(pending))):
                        pending.pop(0)()

                for li in range(NL):
                    blk, j = li // 3, li % 3
                    i = (inp0 + blk) % 3  # block input (residual) buffer
                    if j == 0:
                        src, dst = i, (i + 1) % 3
                    elif j == 1:
                        src, dst = (i + 1) % 3, (i + 2) % 3
                    else:
                        src, dst = (i + 2) % 3, (i + 1) % 3
                    h_in, h_out, r_buf = hb[src], hb[dst], hb[i]
                    ln_here = j == 2 and blk < 5
                    if c == 0:
                        gd_t = tiles[4]
                        w_t = build_w(tiles, wtmps, li)
                        nc.sync.dma_start(w2_d[li, :, :, :], w_t[:, :, :])
                        if li < NL - 1:
                            tiles_nxt = load_layer(li + 1)
                    else:
                        w_t, gd_t = w_nxt, gd_nxt
                        if li < NL - 1:
                            w_nxt = wp.tile([128, KT, DIM], f16, tag="wt",
                                            name="w_pre")
                            nc.sync.dma_start(
                                w_nxt[:, :, :], w2_d[li + 1, :, :, :]
                            )
                            gd_nxt = None
                            if (li + 1) % 3 == 2 and (li + 1) // 3 >= 1:
                                gd_nxt = wp.tile([128, KT, 128], f16,
                                                 tag="gd", name="gd_pre")
                                nc.sync.dma_start(
                                    gd_nxt[:, :, :],
                                    gd_d[(li + 1) // 3 - 1, :, :, :],
                                )

                    def emit_mains(nt):
                        cols = bass.ts(nt, NT)
                        sacc = []
                        s12 = None
                        if ln_here:
                            s12 = ps.tile([33, NT], f32, tag="s12", bufs=1)
                        for ot in range(KT):
                            y_ps = ps.tile([128, NT], f32, tag="y", bufs=3)
                            for kt in range(KT):
                                nc.tensor.matmul(
                                    y_ps[:, :],
                                    lhsT=w_t[:, kt, bass.ts(ot, 128)],
                                    rhs=h_in[:, kt, cols],
                                    start=(kt == 0),
                                    stop=(kt == KT - 1 and j != 2),
                                )
                            if j == 2:
                                # residual (diag(gamma) for blocks >= 1)
                                nc.tensor.matmul(
                                    y_ps[:, :],
                                    lhsT=ident_t[:, :] if blk == 0
                                    else gd_t[:, ot, :],
                                    rhs=r_buf[:, ot, cols],
                                    start=False,
                                    stop=True,
                                )
                            drip()
                            nc.scalar.activation(
                                h_out[:, ot, cols],
                                y_ps[:, :],
                                AF.Relu if j < 2 else AF.Identity,
                                bias=bias_t[:, li, ot : ot + 1],
                            )
                            if ln_here:
                                hsq = sp.tile([128, NT], f16, tag="hsq", bufs=3)
                                nc.scalar.square(hsq[:, :], h_out[:, ot, cols])
                                sacc.append((h_out[:, ot, cols], hsq))
                                # lag-one-ot stats so the PE never waits
                                # on the activation/square writes
                                if ot >= 1:
                                    ho_p, hq_p = sacc[ot - 1]
                                    nc.tensor.matmul(
                                        s12[0:1, :], lhsT=ones_col, rhs=ho_p,
                                        start=(ot == 1), stop=False,
                                    )
                                    nc.tensor.matmul(
                                        s12[32:33, :], lhsT=ones_col,
                                        rhs=hq_p[:, :],
                                        start=(ot == 1), stop=False,
                                    )
                        if ln_here:
                            ho_p, hq_p = sacc[KT - 1]
                            nc.tensor.matmul(
                                s12[0:1, :], lhsT=ones_col, rhs=ho_p,
                                start=False, stop=True,
                            )
                            nc.tensor.matmul(
                                s12[32:33, :], lhsT=ones_col, rhs=hq_p[:, :],
                                start=False, stop=True,
                            )
                            # inv-std chain (small ops, off the PE)
                            m_sb = sp.tile([1, NT], f32, tag="m", bufs=1)
                            nc.vector.tensor_scalar(
                                m_sb[:, :], s12[0:1, :], 1.0 / DIM, None,
                                Alu.mult,
                            )
                            msq = sp.tile([1, NT], f32, tag="msq", bufs=1)
                            nc.vector.tensor_mul(msq[:, :], m_sb[:, :], m_sb[:, :])
                            ve = sp.tile([1, NT], f32, tag="ve", bufs=1)
                            nc.vector.scalar_tensor_tensor(
                                ve[:, :], s12[32:33, :], 1.0 / DIM, msq[:, :],
                                Alu.mult, Alu.subtract,
                            )
                            is_sb = sp.tile([1, NT], f16, tag="isb", bufs=2)
                            if USE_RSQRT:
                                rsqrt(is_sb[:, :], ve[:, :], eps_t[:, :])
                            else:
                                lnv = sp.tile([1, NT], f32, tag="lnv", bufs=1)
                                nc.scalar.activation(
                                    lnv[:, :], ve[:, :], AF.Ln, bias=eps_t[:, :]
                                )
                                nc.scalar.activation(
                                    is_sb[:, :], lnv[:, :], AF.Exp, scale=-0.5
                                )
                            mis = sp.tile([1, NT], f16, tag="mis", bufs=2)
                            nc.vector.tensor_mul(
                                mis[:, :], m_sb[:, :], is_sb[:, :]
                            )

                            boxes = []

                            def step(kt, cols=cols, is_sb=is_sb, mis=mis,
                                     blk=blk, h_out=h_out, boxes=boxes):
                                if kt == 0:
                                    ib = ps.tile([128, NT], f32, tag="ib",
                                                 bufs=1)
                                    nc.tensor.matmul(
                                        ib[:, :], lhsT=ones_row,
                                        rhs=is_sb[:, :],
                                        start=True, stop=True,
                                    )
                                    mib = ps.tile([128, NT], f32, tag="mib",
                                                  bufs=1)
                                    nc.tensor.matmul(
                                        mib[:, :], lhsT=ones_row,
                                        rhs=mis[:, :],
                                        start=True, stop=True,
                                    )
                                    boxes.extend([ib, mib])
                                ib, mib = boxes
                                nc.vector.tensor_mul(
                                    h_out[:, kt, cols], h_out[:, kt, cols],
                                    ib[:, :],
                                )
                                # h = (h*ib + beta/gamma[p]) - m*is[n]
                                nc.vector.scalar_tensor_tensor(
                                    h_out[:, kt, cols], h_out[:, kt, cols],
                                    bog_t[:, blk, kt : kt + 1], mib[:, :],
                                    Alu.add, Alu.subtract,
                                )

                            for kt in range(KT):
                                pending.append(
                                    lambda kt=kt, step=step: step(kt)
                                )

                    for nt in range(NTILES):
                        emit_mains(nt)
                    if c == 0 and li < NL - 1:
                        wtmps = dequant(tiles_nxt, li + 1)
                        tiles = tiles_nxt

                drip(len(pending))
                h_fin = hb[(inp0 + 5 + 1) % 3]
                for kt in range(KT):
                    nc.sync.dma_start(
                        y_d[:, kt, bass.ts(c, CHUNK)], h_fin[:, kt, :]
                    )

    nc.compile()
    return nc


def prep_inputs(x, wq, scales, bias, lora_a, lora_b, gamma, beta):
    """Host-side layout prep; returns per-core input maps."""
    wqc = wq.transpose(0, 2, 1).astype(np.int8) - 8  # [l, i, o] centered
    wqc = wqc.reshape(NL, KT, 128, DIM).transpose(0, 2, 1, 3).copy()  # [l,p,kt,o]

    G = scales.reshape(NL, DIM, DIM // GROUP)  # [l, o, gi]
    p_idx = np.arange(128)[:, None] // GROUP  # [128,1]
    kt_idx = np.arange(KT)[None, :] * (128 // GROUP)  # [1,8]
    gidx = p_idx + kt_idx  # [128, KT] group row index
    srep = G.transpose(0, 2, 1)[:, gidx, :].astype(F16).copy()  # [l,128,8,o]

    la_r = np.ascontiguousarray(lora_a.reshape(NL, RANK, KT, 128)).astype(F16)
    lb_r = np.ascontiguousarray(lora_b.transpose(0, 2, 1)).astype(F16)  # [l, r, o]

    bias_pp = bias.reshape(NL, KT, 128).transpose(2, 0, 1).astype(np.float32).copy()
    gamma_pp = gamma.reshape(5, KT, 128).transpose(2, 0, 1).astype(np.float32).copy()

    # beta/gamma (0 where gamma == 0), per-partition layout [128, 5, KT]
    gsafe = np.where(gamma == 0.0, 1.0, gamma)
    bog = np.where(gamma == 0.0, 0.0, beta / gsafe).astype(np.float32)  # [5, DIM]
    bog_pp = bog.reshape(5, KT, 128).transpose(2, 0, 1).astype(np.float32).copy()

    # diag(gamma) residual weights, partition-major: gdiag[b, p, kt, m] =
    # gamma[b, kt*128+p] if p == m else 0
    gdiag = np.zeros((5, 128, KT, 128), np.float32)
    idx = np.arange(128)
    gdiag[:, idx, :, idx] = gamma.reshape(5, KT, 128).transpose(2, 0, 1)
    gdiag = gdiag.astype(F16)

    shared = {
        "wqc": wqc, "srep": srep, "la_r": la_r, "lb_r": lb_r,
        "bias_pp": bias_pp, "gamma_pp": gamma_pp,
        "bog_pp": bog_pp, "gdiag": gdiag,
        "ones": np.ones((128, 128), F16),
        "ident": np.eye(128, dtype=F16),
    }
    in_maps = []
    for c in range(x.shape[0] // RPC):
        xs = x[c * RPC : (c + 1) * RPC]  # [rows, 1024]
        x_t = np.ascontiguousarray(
            xs.T.reshape(KT, 128, RPC).transpose(1, 0, 2)
        ).astype(F16)
        in_maps.append({"x_t": x_t, **shared})
    return in_maps


def unshard_output(results):
    outs = []
    for r in results:
        y_t = np.asarray(r["y_t"]).reshape(128, KT, RPC)
        outs.append(y_t.transpose(2, 1, 0).reshape(RPC, DIM))
    return np.ascontiguousarray(np.concatenate(outs, axis=0), dtype=np.float32)


def kernel(x, wq, scales, bias, lora_a, lora_b, gamma, beta):
    x, wq, scales, bias, lora_a, lora_b, gamma, beta = (
        np.asarray(a) for a in (x, wq, scales, bias, lora_a, lora_b, gamma, beta)
    )
    nc = build_kernel()
    in_maps = prep_inputs(x, wq, scales, bias, lora_a, lora_b, gamma, beta)
    res = run_bass_kernel_spmd(nc, in_maps, list(range(N_CORES)))
    return unshard_output(res.results)
